# revision 49
# baseline (speedup 1.0000x reference)
"""AttentionBlock (GroupNorm -> qkv -> 8-head attention -> proj -> residual)
as a distributed Bass/Tile kernel on 8 TRN2 NeuronCores.

Sharding: pure data-parallel over batch B=8 -> one batch element per core,
zero collectives. Each core computes its whole attention block.

Per-core algorithm (C=512, L=1024, NH=8, ch=64, G=32 groups):
  - GroupNorm stats via bn_stats per channel + tiny PE matmuls to reduce
    channel stats to group stats (16 channels/group) and broadcast back.
    gamma/beta are folded into the qkv weights host-side, attention scale
    (ch^-1/4 on q and k) is folded into the q weights as 1/sqrt(ch).
  - qkv as channel matmuls in bf16. q,k produced in natural [c, l] layout;
    v produced directly transposed ([l, c] layout) by swapping matmul
    operands, so attention needs no on-chip transposes at all.
  - scores computed TRANSPOSED: sT[s, t] = k^T q (lhsT=k, rhs=q), softmax
    denominator via an extra ones-column appended to v^T (row 64 of the AV
    output accumulates sum_s P[s, t]).  exp on ScalarE from PSUM -> bf16.
  - AV: a[c, t] = (vT|1)^T @ P accumulated over 8 s-chunks.
  - softmax 1/D: AV copied to bf16 SBUF (frees PSUM fast), the D row is
    retiled [1,1024]->[8,128] by DMA so DVE's per-lane iterative reciprocal
    costs 0.8us instead of 6.5us, then a DRAM-bounce DMA broadcasts it
    across partitions for the scale multiply.
  - software pipeline: the whole kernel is emitted as one j-loop over the
    4 head pairs; per s-chunk the PE does QK(j,st) + AV(j,st-1) plus a
    "filler" (a v chunk or the next pair's q/k tile) so the PE stays busy
    while ScalarE grinds the 64 [128,1024] exps (ScalarE ~72us, PE ~103us
    busy -> PE-bound). proj runs kc-outer so its first 3 contraction
    chunks overlap the last softmax-normalization chain.
Wall: ~211us (prev session) -> ~148us measured via the differential
For_i loop; single-shot NTFF ~146-150us.
"""

import sys
import types

import numpy as np
import ml_dtypes

BF16 = ml_dtypes.bfloat16

C = 512
L = 1024
NH = 8
CH = 64
G = 32
EPS = 1e-5
N_CORES = 8


# ---------------------------------------------------------------------------
# Environment compat (inlined so kernel.py is self-contained)
# ---------------------------------------------------------------------------
def _install_compat():
    # 1) NTFF profiling hook shim (image's antenv stub lacks axon_hooks).
    try:
        from antenv.axon_hooks import get_axon_ntff_profile_hook  # noqa: F401
    except ImportError:
        try:
            import antenv
            from trn_agent_boot.trn_boot import _ntff_profile_via_ctypes

            m = types.ModuleType("antenv.axon_hooks")
            m._hook = None
            m.set_axon_ntff_profile_hook = lambda h: setattr(m, "_hook", h)
            m.get_axon_ntff_profile_hook = lambda: m._hook
            sys.modules["antenv.axon_hooks"] = m
            antenv.axon_hooks = m
            m.set_axon_ntff_profile_hook(
                _ntff_profile_via_ctypes("/opt/axon/libaxon_pjrt.so")
            )
        except Exception:
            pass

    # 2) gpsimd.sem_clear over a wide semaphore range exceeds this walrus
    #    build's ISA payload limit ("ISA wrong length"); chunk the clears.
    import concourse.bass as bass

    if not getattr(bass.Bass.clear_and_free_semaphores, "_chunk_patch", False):
        _orig_clear = bass.Bass.clear_and_free_semaphores

        def _chunked_clear(self, sems, _orig=_orig_clear):
            sems = list(sems)
            for i in range(0, len(sems), 4):
                _orig(self, sems[i : i + 4])

        _chunked_clear._chunk_patch = True
        bass.Bass.clear_and_free_semaphores = _chunked_clear



def _split_waits(nc):
    """This walrus build accepts at most ONE semaphore wait per instruction;
    Tile emits up to 2 (and the closing drain more). Split the extras into
    standalone EVENT_SEM instructions inserted just before, on the same
    engine, which is semantically identical (same-engine program order)."""
    from concourse import mybir

    nid = 0
    for blk in nc.m.functions[0].blocks:
        new_list = []
        for inst in blk.instructions:
            si = inst.sync_info
            if si and si.on_wait and len(si.on_wait) > 1:
                waits = list(si.on_wait)
                si.on_wait = waits[-1:]
                for w in waits[:-1]:
                    nid += 1
                    ev = mybir.InstEventSemaphore(
                        name=f"WSPLIT-{nid}", ins=[], outs=[]
                    )
                    ev.engine = inst.engine
                    ev.sync_info = mybir.SyncInfo(on_wait=[w], on_update=[])
                    nc.register_instruction(ev, overwrite=True)
                    new_list.append(ev)
            new_list.append(inst)
        blk.instructions[:] = new_list


# ---------------------------------------------------------------------------
# Bass graph
# ---------------------------------------------------------------------------
def build_nc(loop_n=None):
    import concourse.bass as bass
    import concourse.tile as tile
    from concourse import mybir

    f32 = mybir.dt.float32
    bf = mybir.dt.bfloat16
    AF = mybir.ActivationFunctionType
    OP = mybir.AluOpType

    nc = bass.Bass(trn_type="TRN2")
    xd = nc.declare_dram_parameter("x", [C, L], f32, isOutput=False)
    wqkd = nc.declare_dram_parameter("wqk", [C, 2 * C], bf, isOutput=False)
    wvd = nc.declare_dram_parameter("wv", [C, C], bf, isOutput=False)
    wpd = nc.declare_dram_parameter("wp", [C, C], bf, isOutput=False)
    bqkd = nc.declare_dram_parameter("bqk", [128, 8], f32, isOutput=False)
    bvd = nc.declare_dram_parameter("bvb", [128, C], bf, isOutput=False)
    bpd = nc.declare_dram_parameter("bp", [128, 4], f32, isOutput=False)
    indd = nc.declare_dram_parameter("ind", [128, 8], f32, isOutput=False)
    indTd = nc.declare_dram_parameter("indT", [8, 128], f32, isOutput=False)
    outd = nc.declare_dram_parameter("out", [C, L], f32, isOutput=True)

    with tile.TileContext(nc) as tc:
        with (
            tc.tile_pool(name="cst", bufs=1) as cst,
            tc.tile_pool(name="act", bufs=1) as actp,
            tc.tile_pool(name="ptp", bufs=4) as ptp,
            tc.tile_pool(name="dnp", bufs=2) as dnp,
            tc.tile_pool(name="otp", bufs=2) as otp,
            tc.tile_pool(name="psp", bufs=1, space="PSUM") as psp,
            tc.tile_pool(name="drp", bufs=2, space="DRAM") as drp,
        ):
            # ---- weight/constant tiles (DMAs emitted inside the body AFTER
            # the x loads, so GroupNorm isn't queued behind 3MB of weights)
            wqk_sb = cst.tile([128, 4, 2 * C], bf)
            wv_sb = cst.tile([128, 4, C], bf)
            wp_sb = cst.tile([128, 4, C], bf)
            bqk_sb = cst.tile([128, 8], f32)
            bvb_sb = cst.tile([128, C], bf)
            bp_sb = cst.tile([128, 4], f32)
            ind_sb = cst.tile([128, 8], f32)
            indT_sb = cst.tile([8, 128], f32)
            eps8 = cst.tile([8, 1], f32)
            nc.vector.memset(eps8, EPS)

            def _emit_body():
                import concourse.bass as bass_mod

                # loads split across BOTH HWDGE rings (sync + scalar-queue)
                # so x isn't serialized behind 3MB of weights: x chunks
                # alternate rings; wqk (needed first) rides sync after x,
                # the rest ride the scalar ring
                x_sb = actp.tile([128, 4, L], f32)
                xr = xd[:, :].rearrange("(a p) o -> p a o", p=128)
                # 8 half-chunk loads alternating rings: each bn_stats call
                # only waits for its own 512-col half (subtile deps)
                for t in range(4):
                    for s in range(2):
                        eng = nc.sync if (2 * t + s) % 2 == 0 else nc.scalar
                        eng.dma_start(
                            out=x_sb[:, t, 512 * s : 512 * (s + 1)],
                            in_=xr[:, t, 512 * s : 512 * (s + 1)],
                        )

                nc.sync.dma_start(out=ind_sb, in_=indd[:, :])
                nc.sync.dma_start(out=indT_sb, in_=indTd[:, :])
                nc.sync.dma_start(out=bqk_sb, in_=bqkd[:, :])
                nc.sync.dma_start(
                    out=wqk_sb, in_=wqkd[:, :].rearrange("(a p) o -> p a o", p=128)
                )
                nc.scalar.dma_start(
                    out=wv_sb, in_=wvd[:, :].rearrange("(a p) o -> p a o", p=128)
                )
                nc.scalar.dma_start(out=bvb_sb, in_=bvd[:, :])
                nc.scalar.dma_start(
                    out=wp_sb, in_=wpd[:, :].rearrange("(a p) o -> p a o", p=128)
                )
                nc.scalar.dma_start(out=bp_sb, in_=bpd[:, :])

                xn_sb = actp.tile([128, 4, L], bf)
                q_sb = actp.tile([128, 4, L], bf)
                k_sb = actp.tile([128, 4, L], bf)
                vT_sb = actp.tile([128, 8, NH, CH + 1], bf)
                hid_sb = actp.tile([128, 4, L], bf)
                sc_sb = actp.tile([128, 4, 2], f32)

                # ---- GroupNorm statistics
                st6 = actp.tile([128, 4, 2, 6], f32)
                mv = actp.tile([128, 4, 2], f32)
                stats4 = actp.tile([128, 8], f32)
                for t in range(4):
                    for s in range(2):
                        nc.vector.bn_stats(
                            out=st6[:, t, s, :], in_=x_sb[:, t, 512 * s : 512 * (s + 1)]
                        )
                    nc.vector.bn_aggr(out=mv[:, t, :], in_=st6[:, t, :, :])
                    nc.vector.tensor_copy(
                        out=stats4[:, 2 * t : 2 * t + 1], in_=mv[:, t, 0:1]
                    )
                    # E[x^2] = mean*mean + var in one fused op (AP scalar)
                    nc.vector.scalar_tensor_tensor(
                        out=stats4[:, 2 * t + 1 : 2 * t + 2],
                        in0=mv[:, t, 0:1],
                        scalar=mv[:, t, 0:1],
                        in1=mv[:, t, 1:2],
                        op0=OP.mult,
                        op1=OP.add,
                    )
                gmm = psp.tile([8, 8], f32, tag="B", bufs=2)
                nc.tensor.matmul(gmm, lhsT=ind_sb, rhs=stats4, start=True, stop=True)
                # fused group scale/shift: gmm holds 16*mean (even cols) and
                # 16*E[x^2] (odd cols); two stt ops give var, Ln/Exp give
                # inv_std written straight into gs's strided even columns,
                # and one more stt writes shift = -mean*inv_std to the odd
                # columns — no standalone copies
                # DVE allows only one PSUM operand per instruction: stage the
                # tiny [8, 8] group-stat matmul result in SBUF first
                gmraw = actp.tile([8, 8], f32)
                nc.vector.tensor_copy(out=gmraw, in_=gmm)
                gmmr = gmraw.rearrange("g (t s) -> g t s", s=2)
                msq = actp.tile([8, 4], f32)
                nc.vector.scalar_tensor_tensor(
                    out=msq, in0=gmmr[:, :, 0], scalar=1.0 / 256.0,
                    in1=gmmr[:, :, 0], op0=OP.mult, op1=OP.mult,
                )
                gv = actp.tile([8, 4], f32)
                nc.vector.scalar_tensor_tensor(
                    out=gv, in0=gmmr[:, :, 1], scalar=1.0 / 16.0,
                    in1=msq, op0=OP.mult, op1=OP.subtract,
                )
                # rsqrt(v + eps) = exp(-0.5 * ln(v + eps)): Log and Exp share one
                # ACT table set (natural_log_exp_and_others), so the softmax Exp
                # later needs no table switch.
                sd = actp.tile([8, 4], f32)
                nc.scalar.activation(out=sd, in_=gv, func=AF.Ln, bias=eps8, scale=1.0)
                gs = actp.tile([8, 8], f32)
                gsr = gs.rearrange("g (t s) -> g t s", s=2)
                nc.scalar.activation(out=gsr[:, :, 0], in_=sd, func=AF.Exp, scale=-0.5)
                nc.vector.scalar_tensor_tensor(
                    out=gsr[:, :, 1], in0=gmmr[:, :, 0], scalar=-1.0 / 16.0,
                    in1=gsr[:, :, 0], op0=OP.mult, op1=OP.mult,
                )
                nb = psp.tile([128, 8], f32, tag="B", bufs=2)
                nc.tensor.matmul(nb, lhsT=indT_sb, rhs=gs, start=True, stop=True)
                nc.vector.tensor_copy(
                    out=sc_sb.rearrange("p t s -> p (t s)"), in_=nb
                )
                # normalize: split across DVE and ScalarE (Identity with
                # per-partition scale/bias APs) so the two halves run
                # concurrently — this is on the critical path to the first exp
                for t in range(4):
                    if t % 2 == 0:
                        nc.vector.tensor_scalar(
                            out=xn_sb[:, t, :],
                            in0=x_sb[:, t, :],
                            scalar1=sc_sb[:, t, 0:1],
                            scalar2=sc_sb[:, t, 1:2],
                            op0=OP.mult,
                            op1=OP.add,
                        )
                    else:
                        nc.scalar.activation(
                            out=xn_sb[:, t, :],
                            in_=x_sb[:, t, :],
                            func=AF.Identity,
                            bias=sc_sb[:, t, 1:2],
                            scale=sc_sb[:, t, 0:1],
                        )

                # ---- emission helpers for the software pipeline ----------
                def emit_qk_tile(m):
                    # one m-tile of the q/k matmul, kc-outer so each wqk
                    # stationary block is loaded once for both nh halves;
                    # shares tag-A PSUM slots with the score tiles
                    ps = psp.tile([128, L], f32, tag="A", bufs=2, name=f"qkv{m}")
                    for kc in range(4):
                        for nh in range(2):
                            nc.tensor.matmul(
                                ps[:, 512 * nh : 512 * (nh + 1)],
                                lhsT=wqk_sb[:, kc, 128 * m : 128 * (m + 1)],
                                rhs=xn_sb[:, kc, 512 * nh : 512 * (nh + 1)],
                                start=(kc == 0),
                                stop=(kc == 3),
                                skip_group_check=True,
                            )
                    dst = q_sb if m < 4 else k_sb
                    if m == 4:
                        # pair 0's k bias on ScalarE (idle pre-exp) so the
                        # q and k bias adds run concurrently — this is on
                        # the critical path to the first exp
                        nc.scalar.activation(
                            out=dst[:, 0, :],
                            in_=ps,
                            func=AF.Identity,
                            bias=bqk_sb[:, m : m + 1],
                            scale=1.0,
                        )
                    else:
                        nc.vector.tensor_scalar_add(
                            out=dst[:, m % 4, :], in0=ps,
                            scalar1=bqk_sb[:, m : m + 1],
                        )

                def emit_v_chunk(lt, tag="A"):
                    # vT[l, c] for one 128-l chunk (transposed v via swapped
                    # matmul operands)
                    ps = psp.tile([128, C], f32, tag=tag, bufs=2, name=f"v{lt}")
                    for kc in range(4):
                        nc.tensor.matmul(
                            ps,
                            lhsT=xn_sb[:, kc, 128 * lt : 128 * (lt + 1)],
                            rhs=wv_sb[:, kc, :],
                            start=(kc == 0),
                            stop=(kc == 3),
                            skip_group_check=True,
                        )
                    nc.vector.tensor_tensor(
                        out=vT_sb[:, lt, :, 0:CH],
                        in0=ps.rearrange("p (h c) -> p h c", h=NH),
                        in1=bvb_sb.rearrange("p (h c) -> p h c", h=NH),
                        op=OP.add,
                    )

                pts_t = {}
                av_t = {}

                def emit_qk_chunk(j, st):
                    # scores^T [s-chunk, t] for head pair j + exp on ScalarE
                    # head 1 first throughout (QK -> exp -> AV all consume in
                    # the same order, so no cross-head waiting per chunk)
                    pss = {
                        hh: psp.tile(
                            [128, L], f32, tag="A", bufs=2, name=f"qkt{j}_{st}_{hh}"
                        )
                        for hh in (1, 0)
                    }
                    for hh in (1, 0):
                        po = 64 * hh
                        for nh in range(2):
                            nc.tensor.matmul(
                                pss[hh][:, 512 * nh : 512 * (nh + 1)],
                                lhsT=k_sb[po : po + 64, j, 128 * st : 128 * (st + 1)],
                                rhs=q_sb[po : po + 64, j, 512 * nh : 512 * (nh + 1)],
                                start=True,
                                stop=True,
                                skip_group_check=True,
                            )
                    for hh in (1, 0):
                        nc.scalar.activation(
                            out=pts_t[j][hh][:, st, :], in_=pss[hh], func=AF.Exp
                        )

                def emit_av_chunk(j, st):
                    # accumulate (vT|1)^T @ P for both heads of pair j
                    # (head 1 first: its norm chain has the extra
                    # partition-move DMA, so it should finish first)
                    for hh in (1, 0):
                        for nh in range(2):
                            nc.tensor.matmul(
                                av_t[j][hh][:, 512 * nh : 512 * (nh + 1)],
                                lhsT=vT_sb[:, st, 2 * j + hh, :],
                                rhs=pts_t[j][hh][:, st, 512 * nh : 512 * (nh + 1)],
                                start=(st == 0),
                                stop=(st == 7),
                                skip_group_check=True,
                            )

                def emit_norm(j):
                    # Softmax normalization. First copy each head's AV (incl.
                    # the D row) out of PSUM into bf16 SBUF — this frees the
                    # PSUM B slots immediately so the next pair's AV can
                    # accumulate while the 1/D chain below is in flight.
                    ah = {}
                    for hh in (1, 0):
                        a = dnp.tile([CH + 1, L], bf, tag=f"ah{hh}")
                        if j == 3 and hh == 0:
                            # last pair: ScalarE is idle (all exps done) —
                            # run this copy there so both casts parallelize
                            # on the tail critical path
                            nc.scalar.copy(out=a, in_=av_t[j][hh])
                        else:
                            nc.vector.tensor_copy(out=a, in_=av_t[j][hh])
                        ah[hh] = a
                    # DVE's iterative-divide reciprocal costs ~6 cyc/elem PER
                    # LANE (free-size bound): [1, 1024] would be 6.5us. Retile
                    # both D rows to [16, 128] via SBUF->SBUF DMA so 16 lanes
                    # share the work (~0.8us). NB: keep the source partition
                    # dim at 1 — splitting the free dim into the AP's
                    # partition slot would read physical partitions 1..7.
                    # per-head chains (hh=1 first: it has the extra partition-
                    # move DMA) so each head's recip/bounce overlaps the
                    # other's copy
                    for hh in (1, 0):
                        # for the last pair, head 0's chain rides the (now
                        # idle) scalar-queue HWDGE ring so the two heads'
                        # bounce DMAs fully parallelize on the tail
                        dq = nc.scalar if (j == 3 and hh == 0) else nc.sync
                        dT = dnp.tile([8, 128], bf, tag=f"dT{hh}")
                        dq.dma_start(
                            out=dT,
                            in_=ah[hh][CH : CH + 1, :].rearrange(
                                "o (p a) -> o p a", p=8
                            ),
                        )
                        rT = dnp.tile([8, 128], bf, tag=f"rT{hh}")
                        with nc.allow_low_precision(
                            reason="softmax 1/D in bf16 is within tolerance"
                        ):
                            nc.vector.reciprocal(out=rT, in_=dT)
                        ddr = drp.tile([1, L], bf, tag=f"ddr{hh}")
                        dq.dma_start(
                            out=ddr[:, :].rearrange("o (p a) -> (o p) a", p=8),
                            in_=rT,
                        )
                        dbb = dnp.tile([CH, L], bf, tag=f"dbb{hh}")
                        bcast = bass_mod.AP(
                            tensor=ddr[:, :].tensor,
                            offset=ddr[:, :].offset,
                            ap=[[0, CH]] + list(ddr[:, :].ap[1:]),
                        )
                        dq.dma_start(out=dbb, in_=bcast)
                        # half-granularity so proj's kc=3 matmuls (subtile
                        # deps on hid) can start on the first 512 columns
                        # while the second half is still being scaled
                        if hh == 0:
                            for nh in range(2):
                                sl = slice(512 * nh, 512 * (nh + 1))
                                nc.vector.tensor_mul(
                                    out=hid_sb[0:CH, j, sl],
                                    in0=ah[0][0:CH, sl],
                                    in1=dbb[:, sl],
                                )
                        else:
                            tmpo = dnp.tile([CH, L], bf, tag="tmpo")
                            for nh in range(2):
                                sl = slice(512 * nh, 512 * (nh + 1))
                                nc.vector.tensor_mul(
                                    out=tmpo[:, sl], in0=ah[1][0:CH, sl],
                                    in1=dbb[:, sl],
                                )
                                nc.sync.dma_start(
                                    out=hid_sb[CH:128, j, sl], in_=tmpo[:, sl]
                                )

                def alloc_pts(j):
                    pts_t[j] = [
                        ptp.tile([128, 8, L], bf, tag="pt", name=f"pt{j}_0"),
                        ptp.tile([128, 8, L], bf, tag="pt", name=f"pt{j}_1"),
                    ]

                def alloc_av(j):
                    av_t[j] = [
                        psp.tile([CH + 1, L], f32, tag="B", bufs=2, name=f"av{j}_0"),
                        psp.tile([CH + 1, L], f32, tag="B", bufs=2, name=f"av{j}_1"),
                    ]

                # ---- software pipeline --------------------------------------
                # PE order: q/k for pair 0 first so exp (the ScalarE
                # bottleneck) starts ASAP; v chunks + later q/k pairs + the
                # previous pair's AV fill PE slack inside each st loop, so
                # ScalarE never starves and PE never idles long.
                nc.vector.memset(vT_sb[:, :, :, CH : CH + 1], 1.0)
                # pair 0's q and k tiles interleaved per kc chunk so both
                # finish right after the last xn chunk lands (instead of the
                # k tile serializing 1.7us behind the q tile) — this gates
                # the first exp
                ps0 = psp.tile([128, L], f32, tag="A", bufs=2, name="qkv0")
                ps4 = psp.tile([128, L], f32, tag="A", bufs=2, name="qkv4")
                for kc in range(4):
                    for ps, m in ((ps0, 0), (ps4, 4)):
                        for nh in range(2):
                            nc.tensor.matmul(
                                ps[:, 512 * nh : 512 * (nh + 1)],
                                lhsT=wqk_sb[:, kc, 128 * m : 128 * (m + 1)],
                                rhs=xn_sb[:, kc, 512 * nh : 512 * (nh + 1)],
                                start=(kc == 0),
                                stop=(kc == 3),
                                skip_group_check=True,
                            )
                nc.vector.tensor_scalar_add(
                    out=q_sb[:, 0, :], in0=ps0, scalar1=bqk_sb[:, 0:1]
                )
                nc.scalar.activation(
                    out=k_sb[:, 0, :], in_=ps4, func=AF.Identity,
                    bias=bqk_sb[:, 4:5], scale=1.0,
                )

                # fillers[j][st] -> list of zero-arg emitters
                def qkt(m):
                    return lambda: emit_qk_tile(m)

                def vch(lt):
                    return lambda: emit_v_chunk(lt)

                # v chunk lt must be emitted before AV(j=0, lt); q/k tiles
                # for pair j+1 must be emitted before QK(j+1, 0)
                # v0/v1 ride the briefly-free tag-B PSUM slots (GN tiles
                # vacated, av(0) isn't written until st=1), so the j0
                # pipe-fill isn't throttled by the exp-paced tag-A rotation
                def vchB(lt):
                    return lambda: emit_v_chunk(lt, tag="B")

                fillers = {
                    0: {
                        0: [vchB(0), vchB(1)],
                        1: [vch(2)], 2: [vch(3)], 3: [vch(4)],
                        4: [vch(5)], 5: [vch(6), qkt(1)],
                        6: [vch(7), qkt(5)],
                    },
                    1: {4: [qkt(2)], 6: [qkt(6)]},
                    2: {4: [qkt(3)], 6: [qkt(7)]},
                    3: {},
                }

                for j in range(4):
                    alloc_pts(j)
                    alloc_av(j)
                    for st in range(8):
                        # AV first: it accumulates into already-held PSUM
                        # (no slot wait), so the in-order PE absorbs the
                        # exp-paced tag-A rotation wait behind useful work
                        if st > 0:
                            emit_av_chunk(j, st - 1)
                        emit_qk_chunk(j, st)
                        for f in fillers[j].get(st, []):
                            f()
                    emit_av_chunk(j, 7)
                    emit_norm(j)
                    if j == 0:
                        # residual base: x + b_proj (in place), off the
                        # critical path while ScalarE grinds exps
                        for m in range(4):
                            nc.vector.tensor_scalar_add(
                                out=x_sb[:, m, :],
                                in0=x_sb[:, m, :],
                                scalar1=bp_sb[:, m : m + 1],
                            )

                # ---- proj + residual, kc-outer across all 4 m-tiles: the
                # kc<3 partials only need hid pairs 0..2 and run during the
                # last norm chain; only the kc=3 matmuls wait on hid[:, 3].
                # m2/m3 live in tag-B PSUM slots freed by the norm(3) copies.
                pst = []
                for m in range(4):
                    pst.append(
                        psp.tile(
                            [128, L], f32, tag=("A" if m < 2 else "B"),
                            bufs=2, name=f"proj{m}",
                        )
                    )
                for kc in range(4):
                    for m in range(4):
                        for nh in range(2):
                            nc.tensor.matmul(
                                pst[m][:, 512 * nh : 512 * (nh + 1)],
                                lhsT=wp_sb[:, kc, 128 * m : 128 * (m + 1)],
                                rhs=hid_sb[:, kc, 512 * nh : 512 * (nh + 1)],
                                start=(kc == 0),
                                stop=(kc == 3),
                                skip_group_check=True,
                            )
                # residual add in halves so each half's store DMA issues
                # while DVE works on the next half
                for m in range(4):
                    ob = otp.tile([128, L], f32, tag="ob")
                    for nh in range(2):
                        sl = slice(512 * nh, 512 * (nh + 1))
                        nc.vector.tensor_add(
                            out=ob[:, sl], in0=pst[m][:, sl], in1=x_sb[:, m, sl]
                        )
                        nc.sync.dma_start(
                            out=outd[128 * m : 128 * (m + 1), sl], in_=ob[:, sl]
                        )

            if loop_n:
                with tc.For_i(0, loop_n, 1):
                    _emit_body()
            else:
                _emit_body()

    _split_waits(nc)
    return nc


_NC = None


def _get_nc():
    global _NC
    if _NC is None:
        _install_compat()
        _NC = build_nc()
    return _NC


def _host_prep(x, gamma, beta, w_qkv, b_qkv, w_proj, b_proj):
    x = np.asarray(x, np.float32)
    gamma = np.asarray(gamma, np.float32)
    beta = np.asarray(beta, np.float32)
    w_qkv = np.asarray(w_qkv, np.float32)
    b_qkv = np.asarray(b_qkv, np.float32)
    w_proj = np.asarray(w_proj, np.float32)
    b_proj = np.asarray(b_proj, np.float32)

    s2 = 1.0 / np.sqrt(CH)  # attention scale applied to q AND k => s^2 on q
    Wg = w_qkv * gamma[None, :]
    bb = w_qkv @ beta + b_qkv
    Wg = Wg.copy()
    Wg[0:C] *= s2
    bb = bb.copy()
    bb[0:C] *= s2

    shared = {
        "wqk": np.ascontiguousarray(Wg[0 : 2 * C].T).astype(BF16),
        "wv": np.ascontiguousarray(Wg[2 * C : 3 * C].T).astype(BF16),
        "wp": np.ascontiguousarray(w_proj.T).astype(BF16),
        "bqk": np.ascontiguousarray(bb[0 : 2 * C].reshape(8, 128).T).astype(
            np.float32
        ),
        "bvb": np.broadcast_to(bb[2 * C : 3 * C].reshape(1, C), (128, C)).astype(
            BF16
        ),
        "bp": np.ascontiguousarray(b_proj.reshape(4, 128).T).astype(np.float32),
        "ind": (np.arange(128)[:, None] // 16 == np.arange(8)[None, :]).astype(
            np.float32
        ),
        "indT": (np.arange(128)[None, :] // 16 == np.arange(8)[:, None]).astype(
            np.float32
        ),
    }
    in_maps = []
    for b in range(N_CORES):
        m = dict(shared)
        m["x"] = np.ascontiguousarray(x[b].reshape(C, L))
        in_maps.append(m)
    return in_maps


def run_spmd(in_maps, trace=False):
    from concourse.bass_utils import run_bass_kernel_spmd

    nc = _get_nc()
    return run_bass_kernel_spmd(
        nc, in_maps, core_ids=list(range(N_CORES)), trace=trace
    )


def kernel(x, gamma, beta, w_qkv, b_qkv, w_proj, b_proj):
    _install_compat()
    in_maps = _host_prep(x, gamma, beta, w_qkv, b_qkv, w_proj, b_proj)
    res = run_spmd(in_maps, trace=False)
    out = np.stack(
        [res.results[c]["out"].reshape(C, 32, 32) for c in range(N_CORES)]
    ).astype(np.float32)
    return out



# revision 50
# speedup vs baseline: 1.0955x; 1.0955x over previous
"""AttentionBlock (GroupNorm -> qkv -> 8-head attention -> proj -> residual)
as a distributed Bass/Tile kernel on 8 TRN2 NeuronCores.

Sharding: pure data-parallel over batch B=8 -> one batch element per core,
zero collectives. Each core computes its whole attention block.

Per-core algorithm (C=512, L=1024, NH=8, ch=64, G=32 groups):
  - GroupNorm stats via bn_stats per channel + tiny PE matmuls to reduce
    channel stats to group stats (16 channels/group) and broadcast back.
    gamma/beta are folded into the qkv weights host-side, attention scale
    (ch^-1/4 on q and k) is folded into the q weights as 1/sqrt(ch).
  - qkv as channel matmuls in bf16. q,k produced in natural [c, l] layout;
    v produced directly transposed ([l, c] layout) by swapping matmul
    operands, so attention needs no on-chip transposes at all.
  - scores computed TRANSPOSED: sT[s, t] = k^T q (lhsT=k, rhs=q), softmax
    denominator via an extra ones-column appended to v^T (row 64 of the AV
    output accumulates sum_s P[s, t]).  exp on ScalarE from PSUM -> bf16.
  - AV: a[c, t] = (vT|1)^T @ P accumulated over 8 s-chunks.
  - softmax 1/D: AV copied to bf16 SBUF (frees PSUM fast), the D row is
    retiled [1,1024]->[8,128] by DMA so DVE's per-lane iterative reciprocal
    costs 0.8us instead of 6.5us, then a DRAM-bounce DMA broadcasts it
    across partitions for the scale multiply.
  - software pipeline: the whole kernel is emitted as one j-loop over the
    4 head pairs; per s-chunk the PE does QK(j,st) + AV(j,st-1) plus a
    "filler" (a v chunk or the next pair's q/k tile) so the PE stays busy
    while ScalarE grinds the 64 [128,1024] exps (ScalarE ~72us, PE ~103us
    busy -> PE-bound). proj runs kc-outer so its first 3 contraction
    chunks overlap the last softmax-normalization chain.
Wall: ~211us (prev session) -> ~148us measured via the differential
For_i loop; single-shot NTFF ~146-150us.
"""

import sys
import types

import numpy as np
import ml_dtypes

BF16 = ml_dtypes.bfloat16

C = 512
L = 1024
NH = 8
CH = 64
G = 32
EPS = 1e-5
N_CORES = 8


# ---------------------------------------------------------------------------
# Environment compat (inlined so kernel.py is self-contained)
# ---------------------------------------------------------------------------
def _install_compat():
    # 1) NTFF profiling hook shim (image's antenv stub lacks axon_hooks).
    try:
        from antenv.axon_hooks import get_axon_ntff_profile_hook  # noqa: F401
    except ImportError:
        try:
            import antenv
            from trn_agent_boot.trn_boot import _ntff_profile_via_ctypes

            m = types.ModuleType("antenv.axon_hooks")
            m._hook = None
            m.set_axon_ntff_profile_hook = lambda h: setattr(m, "_hook", h)
            m.get_axon_ntff_profile_hook = lambda: m._hook
            sys.modules["antenv.axon_hooks"] = m
            antenv.axon_hooks = m
            m.set_axon_ntff_profile_hook(
                _ntff_profile_via_ctypes("/opt/axon/libaxon_pjrt.so")
            )
        except Exception:
            pass

    # 2) gpsimd.sem_clear over a wide semaphore range exceeds this walrus
    #    build's ISA payload limit ("ISA wrong length"); chunk the clears.
    import concourse.bass as bass

    if not getattr(bass.Bass.clear_and_free_semaphores, "_chunk_patch", False):
        _orig_clear = bass.Bass.clear_and_free_semaphores

        def _chunked_clear(self, sems, _orig=_orig_clear):
            sems = list(sems)
            for i in range(0, len(sems), 4):
                _orig(self, sems[i : i + 4])

        _chunked_clear._chunk_patch = True
        bass.Bass.clear_and_free_semaphores = _chunked_clear



def _split_waits(nc):
    """This walrus build accepts at most ONE semaphore wait per instruction;
    Tile emits up to 2 (and the closing drain more). Split the extras into
    standalone EVENT_SEM instructions inserted just before, on the same
    engine, which is semantically identical (same-engine program order)."""
    from concourse import mybir

    nid = 0
    for blk in nc.m.functions[0].blocks:
        new_list = []
        for inst in blk.instructions:
            si = inst.sync_info
            if si and si.on_wait and len(si.on_wait) > 1:
                waits = list(si.on_wait)
                si.on_wait = waits[-1:]
                for w in waits[:-1]:
                    nid += 1
                    ev = mybir.InstEventSemaphore(
                        name=f"WSPLIT-{nid}", ins=[], outs=[]
                    )
                    ev.engine = inst.engine
                    ev.sync_info = mybir.SyncInfo(on_wait=[w], on_update=[])
                    nc.register_instruction(ev, overwrite=True)
                    new_list.append(ev)
            new_list.append(inst)
        blk.instructions[:] = new_list


# ---------------------------------------------------------------------------
# Bass graph
# ---------------------------------------------------------------------------
def build_nc(loop_n=None):
    import concourse.bass as bass
    import concourse.tile as tile
    from concourse import mybir

    f32 = mybir.dt.float32
    bf = mybir.dt.bfloat16
    AF = mybir.ActivationFunctionType
    OP = mybir.AluOpType

    nc = bass.Bass(trn_type="TRN2")
    xd = nc.declare_dram_parameter("x", [C, L], f32, isOutput=False)
    wqkd = nc.declare_dram_parameter("wqk", [C, 2 * C], bf, isOutput=False)
    wvd = nc.declare_dram_parameter("wv", [C, C], bf, isOutput=False)
    wpd = nc.declare_dram_parameter("wp", [C, C], bf, isOutput=False)
    bqkd = nc.declare_dram_parameter("bqk", [128, 8], f32, isOutput=False)
    bvd = nc.declare_dram_parameter("bvb", [128, C], bf, isOutput=False)
    bpd = nc.declare_dram_parameter("bp", [128, 4], f32, isOutput=False)
    indd = nc.declare_dram_parameter("ind", [128, 8], f32, isOutput=False)
    indTd = nc.declare_dram_parameter("indT", [8, 128], f32, isOutput=False)
    outd = nc.declare_dram_parameter("out", [C, L], f32, isOutput=True)

    with tile.TileContext(nc) as tc:
        with (
            tc.tile_pool(name="cst", bufs=1) as cst,
            tc.tile_pool(name="act", bufs=1) as actp,
            tc.tile_pool(name="ptp", bufs=4) as ptp,
            tc.tile_pool(name="dnp", bufs=2) as dnp,
            tc.tile_pool(name="otp", bufs=2) as otp,
            tc.tile_pool(name="psp", bufs=1, space="PSUM") as psp,
            tc.tile_pool(name="drp", bufs=2, space="DRAM") as drp,
        ):
            # ---- weight/constant tiles (DMAs emitted inside the body AFTER
            # the x loads, so GroupNorm isn't queued behind 3MB of weights)
            wqk_sb = cst.tile([128, 4, 2 * C], bf)
            wv_sb = cst.tile([128, 4, C], bf)
            wp_sb = cst.tile([128, 4, C], bf)
            bqk_sb = cst.tile([128, 8], f32)
            bvb_sb = cst.tile([128, C], bf)
            bp_sb = cst.tile([128, 4], f32)
            ind_sb = cst.tile([128, 8], f32)
            indT_sb = cst.tile([8, 128], f32)
            eps8 = cst.tile([8, 1], f32)
            nc.vector.memset(eps8, EPS)

            def _emit_body():
                import concourse.bass as bass_mod

                # loads split across BOTH HWDGE rings (sync + scalar-queue)
                # so x isn't serialized behind 3MB of weights: x chunks
                # alternate rings; wqk (needed first) rides sync after x,
                # the rest ride the scalar ring
                x_sb = actp.tile([128, 4, L], f32)
                xr = xd[:, :].rearrange("(a p) o -> p a o", p=128)
                # 8 half-chunk loads alternating rings: each bn_stats call
                # only waits for its own 512-col half (subtile deps)
                for t in range(4):
                    for s in range(2):
                        eng = nc.sync if (2 * t + s) % 2 == 0 else nc.scalar
                        eng.dma_start(
                            out=x_sb[:, t, 512 * s : 512 * (s + 1)],
                            in_=xr[:, t, 512 * s : 512 * (s + 1)],
                        )

                nc.sync.dma_start(out=ind_sb, in_=indd[:, :])
                nc.sync.dma_start(out=indT_sb, in_=indTd[:, :])
                nc.sync.dma_start(out=bqk_sb, in_=bqkd[:, :])
                nc.sync.dma_start(
                    out=wqk_sb, in_=wqkd[:, :].rearrange("(a p) o -> p a o", p=128)
                )
                nc.scalar.dma_start(
                    out=wv_sb, in_=wvd[:, :].rearrange("(a p) o -> p a o", p=128)
                )
                nc.scalar.dma_start(out=bvb_sb, in_=bvd[:, :])
                nc.scalar.dma_start(
                    out=wp_sb, in_=wpd[:, :].rearrange("(a p) o -> p a o", p=128)
                )
                nc.scalar.dma_start(out=bp_sb, in_=bpd[:, :])

                xn_sb = actp.tile([128, 4, L], bf)
                q_sb = actp.tile([128, 4, L], bf)
                k_sb = actp.tile([128, 4, L], bf)
                vT_sb = actp.tile([128, 8, NH, CH + 1], bf)
                hid_sb = actp.tile([128, 4, L], bf)
                sc_sb = actp.tile([128, 4, 2], f32)

                # ---- GroupNorm statistics
                st6 = actp.tile([128, 4, 2, 6], f32)
                mv = actp.tile([128, 4, 2], f32)
                stats4 = actp.tile([128, 8], f32)
                for t in range(4):
                    for s in range(2):
                        nc.vector.bn_stats(
                            out=st6[:, t, s, :], in_=x_sb[:, t, 512 * s : 512 * (s + 1)]
                        )
                    nc.vector.bn_aggr(out=mv[:, t, :], in_=st6[:, t, :, :])
                    nc.vector.tensor_copy(
                        out=stats4[:, 2 * t : 2 * t + 1], in_=mv[:, t, 0:1]
                    )
                    # E[x^2] = mean*mean + var in one fused op (AP scalar)
                    nc.vector.scalar_tensor_tensor(
                        out=stats4[:, 2 * t + 1 : 2 * t + 2],
                        in0=mv[:, t, 0:1],
                        scalar=mv[:, t, 0:1],
                        in1=mv[:, t, 1:2],
                        op0=OP.mult,
                        op1=OP.add,
                    )
                gmm = psp.tile([8, 8], f32, tag="B", bufs=2)
                nc.tensor.matmul(gmm, lhsT=ind_sb, rhs=stats4, start=True, stop=True)
                # fused group scale/shift: gmm holds 16*mean (even cols) and
                # 16*E[x^2] (odd cols); two stt ops give var, Ln/Exp give
                # inv_std written straight into gs's strided even columns,
                # and one more stt writes shift = -mean*inv_std to the odd
                # columns — no standalone copies
                # DVE allows only one PSUM operand per instruction: stage the
                # tiny [8, 8] group-stat matmul result in SBUF first
                gmraw = actp.tile([8, 8], f32)
                nc.vector.tensor_copy(out=gmraw, in_=gmm)
                gmmr = gmraw.rearrange("g (t s) -> g t s", s=2)
                msq = actp.tile([8, 4], f32)
                nc.vector.scalar_tensor_tensor(
                    out=msq, in0=gmmr[:, :, 0], scalar=1.0 / 256.0,
                    in1=gmmr[:, :, 0], op0=OP.mult, op1=OP.mult,
                )
                gv = actp.tile([8, 4], f32)
                nc.vector.scalar_tensor_tensor(
                    out=gv, in0=gmmr[:, :, 1], scalar=1.0 / 16.0,
                    in1=msq, op0=OP.mult, op1=OP.subtract,
                )
                # rsqrt(v + eps) = exp(-0.5 * ln(v + eps)): Log and Exp share one
                # ACT table set (natural_log_exp_and_others), so the softmax Exp
                # later needs no table switch.
                sd = actp.tile([8, 4], f32)
                nc.scalar.activation(out=sd, in_=gv, func=AF.Ln, bias=eps8, scale=1.0)
                gs = actp.tile([8, 8], f32)
                gsr = gs.rearrange("g (t s) -> g t s", s=2)
                nc.scalar.activation(out=gsr[:, :, 0], in_=sd, func=AF.Exp, scale=-0.5)
                nc.vector.scalar_tensor_tensor(
                    out=gsr[:, :, 1], in0=gmmr[:, :, 0], scalar=-1.0 / 16.0,
                    in1=gsr[:, :, 0], op0=OP.mult, op1=OP.mult,
                )
                nb = psp.tile([128, 8], f32, tag="B", bufs=2)
                nc.tensor.matmul(nb, lhsT=indT_sb, rhs=gs, start=True, stop=True)
                nc.vector.tensor_copy(
                    out=sc_sb.rearrange("p t s -> p (t s)"), in_=nb
                )
                # normalize: split across DVE and ScalarE (Identity with
                # per-partition scale/bias APs) so the two halves run
                # concurrently — this is on the critical path to the first exp
                for t in range(4):
                    if t % 2 == 0:
                        nc.vector.tensor_scalar(
                            out=xn_sb[:, t, :],
                            in0=x_sb[:, t, :],
                            scalar1=sc_sb[:, t, 0:1],
                            scalar2=sc_sb[:, t, 1:2],
                            op0=OP.mult,
                            op1=OP.add,
                        )
                    else:
                        nc.scalar.activation(
                            out=xn_sb[:, t, :],
                            in_=x_sb[:, t, :],
                            func=AF.Identity,
                            bias=sc_sb[:, t, 1:2],
                            scale=sc_sb[:, t, 0:1],
                        )

                # ---- emission helpers for the software pipeline ----------
                def emit_qk_tile(m):
                    # one m-tile of the q/k matmul, kc-outer so each wqk
                    # stationary block is loaded once for both nh halves;
                    # shares tag-A PSUM slots with the score tiles
                    ps = psp.tile([128, L], f32, tag="A", bufs=2, name=f"qkv{m}")
                    for kc in range(4):
                        for nh in range(2):
                            nc.tensor.matmul(
                                ps[:, 512 * nh : 512 * (nh + 1)],
                                lhsT=wqk_sb[:, kc, 128 * m : 128 * (m + 1)],
                                rhs=xn_sb[:, kc, 512 * nh : 512 * (nh + 1)],
                                start=(kc == 0),
                                stop=(kc == 3),
                                skip_group_check=True,
                            )
                    dst = q_sb if m < 4 else k_sb
                    if m == 4:
                        # pair 0's k bias on ScalarE (idle pre-exp) so the
                        # q and k bias adds run concurrently — this is on
                        # the critical path to the first exp
                        nc.scalar.activation(
                            out=dst[:, 0, :],
                            in_=ps,
                            func=AF.Identity,
                            bias=bqk_sb[:, m : m + 1],
                            scale=1.0,
                        )
                    else:
                        nc.vector.tensor_scalar_add(
                            out=dst[:, m % 4, :], in0=ps,
                            scalar1=bqk_sb[:, m : m + 1],
                        )

                def emit_v_chunk(lt, tag="A"):
                    # vT[l, c] for one 128-l chunk (transposed v via swapped
                    # matmul operands)
                    ps = psp.tile([128, C], f32, tag=tag, bufs=2, name=f"v{lt}")
                    for kc in range(4):
                        nc.tensor.matmul(
                            ps,
                            lhsT=xn_sb[:, kc, 128 * lt : 128 * (lt + 1)],
                            rhs=wv_sb[:, kc, :],
                            start=(kc == 0),
                            stop=(kc == 3),
                            skip_group_check=True,
                        )
                    nc.vector.tensor_tensor(
                        out=vT_sb[:, lt, :, 0:CH],
                        in0=ps.rearrange("p (h c) -> p h c", h=NH),
                        in1=bvb_sb.rearrange("p (h c) -> p h c", h=NH),
                        op=OP.add,
                    )

                pts_t = {}
                av_t = {}

                def emit_qk_chunk(j, st):
                    # scores^T [s-chunk, t] for head pair j + exp on ScalarE
                    # head 1 first throughout (QK -> exp -> AV all consume in
                    # the same order, so no cross-head waiting per chunk)
                    pss = {
                        hh: psp.tile(
                            [128, L], f32, tag="A", bufs=2, name=f"qkt{j}_{st}_{hh}"
                        )
                        for hh in (1, 0)
                    }
                    for hh in (1, 0):
                        po = 64 * hh
                        for nh in range(2):
                            nc.tensor.matmul(
                                pss[hh][:, 512 * nh : 512 * (nh + 1)],
                                lhsT=k_sb[po : po + 64, j, 128 * st : 128 * (st + 1)],
                                rhs=q_sb[po : po + 64, j, 512 * nh : 512 * (nh + 1)],
                                start=True,
                                stop=True,
                                skip_group_check=True,
                            )
                    for hh in (1, 0):
                        nc.scalar.activation(
                            out=pts_t[j][hh][:, st, :], in_=pss[hh], func=AF.Exp
                        )

                def emit_av_chunk(j, st):
                    # accumulate (vT|1)^T @ P for both heads of pair j
                    # (head 1 first: its norm chain has the extra
                    # partition-move DMA, so it should finish first)
                    for hh in (1, 0):
                        for nh in range(2):
                            nc.tensor.matmul(
                                av_t[j][hh][:, 512 * nh : 512 * (nh + 1)],
                                lhsT=vT_sb[:, st, 2 * j + hh, :],
                                rhs=pts_t[j][hh][:, st, 512 * nh : 512 * (nh + 1)],
                                start=(st == 0),
                                stop=(st == 7),
                                skip_group_check=True,
                            )

                def emit_norm(j):
                    # Softmax normalization. First copy each head's AV (incl.
                    # the D row) out of PSUM into bf16 SBUF — this frees the
                    # PSUM B slots immediately so the next pair's AV can
                    # accumulate while the 1/D chain below is in flight.
                    ah = {}
                    for hh in (1, 0):
                        a = dnp.tile([CH + 1, L], bf, tag=f"ah{hh}")
                        if j == 3 and hh == 0:
                            # last pair: ScalarE is idle (all exps done) —
                            # run this copy there so both casts parallelize
                            # on the tail critical path
                            nc.scalar.copy(out=a, in_=av_t[j][hh])
                        else:
                            nc.vector.tensor_copy(out=a, in_=av_t[j][hh])
                        ah[hh] = a
                    # DVE's iterative-divide reciprocal costs ~6 cyc/elem PER
                    # LANE (free-size bound): [1, 1024] would be 6.5us. Retile
                    # both D rows to [16, 128] via SBUF->SBUF DMA so 16 lanes
                    # share the work (~0.8us). NB: keep the source partition
                    # dim at 1 — splitting the free dim into the AP's
                    # partition slot would read physical partitions 1..7.
                    # per-head chains (hh=1 first: it has the extra partition-
                    # move DMA) so each head's recip/bounce overlaps the
                    # other's copy
                    for hh in (1, 0):
                        # for the last pair, head 0's chain rides the (now
                        # idle) scalar-queue HWDGE ring so the two heads'
                        # bounce DMAs fully parallelize on the tail
                        dq = nc.scalar if (j == 3 and hh == 0) else nc.sync
                        dT = dnp.tile([8, 128], bf, tag=f"dT{hh}")
                        dq.dma_start(
                            out=dT,
                            in_=ah[hh][CH : CH + 1, :].rearrange(
                                "o (p a) -> o p a", p=8
                            ),
                        )
                        rT = dnp.tile([8, 128], bf, tag=f"rT{hh}")
                        with nc.allow_low_precision(
                            reason="softmax 1/D in bf16 is within tolerance"
                        ):
                            nc.vector.reciprocal(out=rT, in_=dT)
                        ddr = drp.tile([1, L], bf, tag=f"ddr{hh}")
                        dq.dma_start(
                            out=ddr[:, :].rearrange("o (p a) -> (o p) a", p=8),
                            in_=rT,
                        )
                        dbb = dnp.tile([CH, L], bf, tag=f"dbb{hh}")
                        bcast = bass_mod.AP(
                            tensor=ddr[:, :].tensor,
                            offset=ddr[:, :].offset,
                            ap=[[0, CH]] + list(ddr[:, :].ap[1:]),
                        )
                        dq.dma_start(out=dbb, in_=bcast)
                        # half-granularity so proj's kc=3 matmuls (subtile
                        # deps on hid) can start on the first 512 columns
                        # while the second half is still being scaled
                        if hh == 0:
                            for nh in range(2):
                                sl = slice(512 * nh, 512 * (nh + 1))
                                nc.vector.tensor_mul(
                                    out=hid_sb[0:CH, j, sl],
                                    in0=ah[0][0:CH, sl],
                                    in1=dbb[:, sl],
                                )
                        else:
                            tmpo = dnp.tile([CH, L], bf, tag="tmpo")
                            for nh in range(2):
                                sl = slice(512 * nh, 512 * (nh + 1))
                                nc.vector.tensor_mul(
                                    out=tmpo[:, sl], in0=ah[1][0:CH, sl],
                                    in1=dbb[:, sl],
                                )
                                nc.sync.dma_start(
                                    out=hid_sb[CH:128, j, sl], in_=tmpo[:, sl]
                                )

                def alloc_pts(j):
                    pts_t[j] = [
                        ptp.tile([128, 8, L], bf, tag="pt", name=f"pt{j}_0"),
                        ptp.tile([128, 8, L], bf, tag="pt", name=f"pt{j}_1"),
                    ]

                def alloc_av(j):
                    av_t[j] = [
                        psp.tile([CH + 1, L], f32, tag="B", bufs=2, name=f"av{j}_0"),
                        psp.tile([CH + 1, L], f32, tag="B", bufs=2, name=f"av{j}_1"),
                    ]

                # ---- software pipeline --------------------------------------
                # PE order: q/k for pair 0 first so exp (the ScalarE
                # bottleneck) starts ASAP; v chunks + later q/k pairs + the
                # previous pair's AV fill PE slack inside each st loop, so
                # ScalarE never starves and PE never idles long.
                nc.vector.memset(vT_sb[:, :, :, CH : CH + 1], 1.0)
                # pair 0's q and k tiles interleaved per kc chunk so both
                # finish right after the last xn chunk lands (instead of the
                # k tile serializing 1.7us behind the q tile) — this gates
                # the first exp
                ps0 = psp.tile([128, L], f32, tag="A", bufs=2, name="qkv0")
                ps4 = psp.tile([128, L], f32, tag="A", bufs=2, name="qkv4")
                for kc in range(4):
                    for ps, m in ((ps0, 0), (ps4, 4)):
                        for nh in range(2):
                            nc.tensor.matmul(
                                ps[:, 512 * nh : 512 * (nh + 1)],
                                lhsT=wqk_sb[:, kc, 128 * m : 128 * (m + 1)],
                                rhs=xn_sb[:, kc, 512 * nh : 512 * (nh + 1)],
                                start=(kc == 0),
                                stop=(kc == 3),
                                skip_group_check=True,
                            )
                nc.vector.tensor_scalar_add(
                    out=q_sb[:, 0, :], in0=ps0, scalar1=bqk_sb[:, 0:1]
                )
                nc.scalar.activation(
                    out=k_sb[:, 0, :], in_=ps4, func=AF.Identity,
                    bias=bqk_sb[:, 4:5], scale=1.0,
                )

                # fillers[j][st] -> list of zero-arg emitters
                def qkt(m):
                    return lambda: emit_qk_tile(m)

                def vch(lt):
                    return lambda: emit_v_chunk(lt)

                # v chunk lt must be emitted before AV(j=0, lt); q/k tiles
                # for pair j+1 must be emitted before QK(j+1, 0)
                # v0/v1 ride the briefly-free tag-B PSUM slots (GN tiles
                # vacated, av(0) isn't written until st=1), so the j0
                # pipe-fill isn't throttled by the exp-paced tag-A rotation
                def vchB(lt):
                    return lambda: emit_v_chunk(lt, tag="B")

                fillers = {
                    0: {
                        0: [vchB(0), vchB(1)],
                        1: [vch(2)], 2: [vch(3)], 3: [vch(4)],
                        4: [vch(5)], 5: [vch(6), qkt(1)],
                        6: [vch(7), qkt(5)],
                    },
                    1: {4: [qkt(2)], 6: [qkt(6)]},
                    2: {4: [qkt(3)], 6: [qkt(7)]},
                    3: {},
                }

                for j in range(4):
                    alloc_pts(j)
                    alloc_av(j)
                    for st in range(8):
                        emit_qk_chunk(j, st)
                        if st > 0:
                            emit_av_chunk(j, st - 1)
                        for f in fillers[j].get(st, []):
                            f()
                    emit_av_chunk(j, 7)
                    emit_norm(j)
                    if j == 0:
                        # residual base: x + b_proj (in place), off the
                        # critical path while ScalarE grinds exps
                        for m in range(4):
                            nc.vector.tensor_scalar_add(
                                out=x_sb[:, m, :],
                                in0=x_sb[:, m, :],
                                scalar1=bp_sb[:, m : m + 1],
                            )

                # ---- proj + residual, kc-outer across all 4 m-tiles: the
                # kc<3 partials only need hid pairs 0..2 and run during the
                # last norm chain; only the kc=3 matmuls wait on hid[:, 3].
                # m2/m3 live in tag-B PSUM slots freed by the norm(3) copies.
                pst = []
                for m in range(4):
                    pst.append(
                        psp.tile(
                            [128, L], f32, tag=("A" if m < 2 else "B"),
                            bufs=2, name=f"proj{m}",
                        )
                    )
                for kc in range(4):
                    for m in range(4):
                        for nh in range(2):
                            nc.tensor.matmul(
                                pst[m][:, 512 * nh : 512 * (nh + 1)],
                                lhsT=wp_sb[:, kc, 128 * m : 128 * (m + 1)],
                                rhs=hid_sb[:, kc, 512 * nh : 512 * (nh + 1)],
                                start=(kc == 0),
                                stop=(kc == 3),
                                skip_group_check=True,
                            )
                # residual add in halves so each half's store DMA issues
                # while DVE works on the next half
                for m in range(4):
                    ob = otp.tile([128, L], f32, tag="ob")
                    for nh in range(2):
                        sl = slice(512 * nh, 512 * (nh + 1))
                        nc.vector.tensor_add(
                            out=ob[:, sl], in0=pst[m][:, sl], in1=x_sb[:, m, sl]
                        )
                        nc.sync.dma_start(
                            out=outd[128 * m : 128 * (m + 1), sl], in_=ob[:, sl]
                        )

            if loop_n:
                with tc.For_i(0, loop_n, 1):
                    _emit_body()
            else:
                _emit_body()

    _split_waits(nc)
    return nc


_NC = None


def _get_nc():
    global _NC
    if _NC is None:
        _install_compat()
        _NC = build_nc()
    return _NC


def _host_prep(x, gamma, beta, w_qkv, b_qkv, w_proj, b_proj):
    x = np.asarray(x, np.float32)
    gamma = np.asarray(gamma, np.float32)
    beta = np.asarray(beta, np.float32)
    w_qkv = np.asarray(w_qkv, np.float32)
    b_qkv = np.asarray(b_qkv, np.float32)
    w_proj = np.asarray(w_proj, np.float32)
    b_proj = np.asarray(b_proj, np.float32)

    s2 = 1.0 / np.sqrt(CH)  # attention scale applied to q AND k => s^2 on q
    Wg = w_qkv * gamma[None, :]
    bb = w_qkv @ beta + b_qkv
    Wg = Wg.copy()
    Wg[0:C] *= s2
    bb = bb.copy()
    bb[0:C] *= s2

    shared = {
        "wqk": np.ascontiguousarray(Wg[0 : 2 * C].T).astype(BF16),
        "wv": np.ascontiguousarray(Wg[2 * C : 3 * C].T).astype(BF16),
        "wp": np.ascontiguousarray(w_proj.T).astype(BF16),
        "bqk": np.ascontiguousarray(bb[0 : 2 * C].reshape(8, 128).T).astype(
            np.float32
        ),
        "bvb": np.broadcast_to(bb[2 * C : 3 * C].reshape(1, C), (128, C)).astype(
            BF16
        ),
        "bp": np.ascontiguousarray(b_proj.reshape(4, 128).T).astype(np.float32),
        "ind": (np.arange(128)[:, None] // 16 == np.arange(8)[None, :]).astype(
            np.float32
        ),
        "indT": (np.arange(128)[None, :] // 16 == np.arange(8)[:, None]).astype(
            np.float32
        ),
    }
    in_maps = []
    for b in range(N_CORES):
        m = dict(shared)
        m["x"] = np.ascontiguousarray(x[b].reshape(C, L))
        in_maps.append(m)
    return in_maps


def run_spmd(in_maps, trace=False):
    from concourse.bass_utils import run_bass_kernel_spmd

    nc = _get_nc()
    return run_bass_kernel_spmd(
        nc, in_maps, core_ids=list(range(N_CORES)), trace=trace
    )


def kernel(x, gamma, beta, w_qkv, b_qkv, w_proj, b_proj):
    _install_compat()
    in_maps = _host_prep(x, gamma, beta, w_qkv, b_qkv, w_proj, b_proj)
    res = run_spmd(in_maps, trace=False)
    out = np.stack(
        [res.results[c]["out"].reshape(C, 32, 32) for c in range(N_CORES)]
    ).astype(np.float32)
    return out

